# revision 56
# baseline (speedup 1.0000x reference)
"""Trainium2 Bass kernel for BatchMatchedMSELoss.

loss = mean_i min_j mean_d (input[i,d] - target[j,d])^2

Decomposition:
  mse[i,j]  = (||x_i||^2 + ||t_j||^2 - 2<x_i, t_j>) / D
  min_j mse = (||x_i||^2 + min_j(||t_j||^2 - 2<x_i,t_j>)) / D

Sharding: each core owns a 1024-row slice of TARGET (j) and sees ALL input
rows i; j lives on PSUM partitions:
  psum[jp, i] = 2<x_i, t_j>      (fp8 e4m3 DoubleRow matmul, K=256, f32 accum)

The kernel is PSUM-drain-bound: only DVE and ACT can read PSUM (~1.04-1.24
ns/col/op measured).  Each [128, 2048] tile of the cross matrix is drained
CONCURRENTLY by both engines on disjoint 2-bank psum halves allocated from
SEPARATE tile pools (distinct tile objects keep the Tile framework from
serializing the two consumers):
  ACT   cols [0:1024):    activation identity+bias -> fp16 staging, shipped
                          raw to HBM; the host max-folds shipped tiles.
  DVE   cols [1024:2048): fused scalar_tensor_tensor (bias+max into acc).
The DVE half is produced first (its stream is the long pole).  Both engines
run back-to-back at ~1.0-1.2us per tile; steady state ~= 32 tiles x ~1.2us.
Host combines acc partials and shipped tiles, maxes over (core, partition),
adds ||x_i||^2 + C, /D, and means.

Measured on trn2 (8 cores, axon): HW exec ~62-64us (baseline 91us), loss
relative error vs the f32 reference ~1.2e-4.
"""

import os
import sys

sys.path.insert(0, "/opt/trn_rl_repo")

import numpy as np
import ml_dtypes

B = 8192
D = 256
NCORES = 8
JS = B // NCORES  # 1024 target rows (j) per core
P = 128
KC = D // P  # 2 contraction chunks
JT = JS // P  # 8 j-tiles per core
NT = 512  # PSUM bank width in f32
IBW = 2048  # i-block width (one PSUM tile = 4 banks, 2 bufs)
IB = B // IBW  # 4 i-blocks
HS = IBW // NT  # 4 matmul subtiles per (j-tile, i-block)
CS = 1024  # ACT-drained column split: bank-aligned 2+2 banks per engine
DW = IBW - CS  # DVE-drained width (1024)

_CACHE = {}


def _build_nc():
    from contextlib import ExitStack

    import concourse.bacc as bacc
    import concourse.tile as tile
    import concourse.mybir as mybir

    fp16 = mybir.dt.float16
    f32 = mybir.dt.float32

    nc = bacc.Bacc("TRN2", target_bir_lowering=False, debug=False)

    fp8 = mybir.dt.float8e4

    # tgtT: (target_shard).T (fp8 e4m3) [D, JS]; inT: (2*input).T (fp8) [D, B]
    tgtT_d = nc.dram_tensor("tgtT", [D, JS], fp8, kind="ExternalInput").ap()
    inT_d = nc.dram_tensor("inT", [D, B], fp8, kind="ExternalInput").ap()
    # negtg[p, jt] = -(||t_j||^2 - C) for local j = jt*128 + p
    negtg_d = nc.dram_tensor("negtg", [P, JT], f32, kind="ExternalInput").ap()
    # acc partials over the DVE column ranges; host maxes over (core, p)
    out_d = nc.dram_tensor("rowmax", [P, IB * DW], fp16, kind="ExternalOutput").ap()
    # raw biased fp16 tiles (ACT ranges), host-folded: JT*IB slots of [P, CS]
    ship_d = nc.dram_tensor("ship", [P, JT * IB * CS], fp16, kind="ExternalOutput").ap()

    with tile.TileContext(nc) as tc, ExitStack() as ctx:
        persist = ctx.enter_context(tc.tile_pool(name="persist", bufs=1))
        # separate pools for the ACT-drained and DVE-drained halves of each
        # tile: distinct tile objects prevent the Tile framework from
        # serializing the two consumers of one psum tile
        psum_a = ctx.enter_context(tc.tile_pool(name="psum_a", bufs=2, space="PSUM"))
        psum_d = ctx.enter_context(tc.tile_pool(name="psum_d", bufs=2, space="PSUM"))
        m_pool = ctx.enter_context(tc.tile_pool(name="m", bufs=8))

        # --- persistent SBUF buffers ---
        # NOTE: Tile's DMA-write tracking is per-tile-object, not per-range:
        # a reader waits for ALL loads into the tile it touches.  Regions
        # consumed at different times therefore get separate tiles (jt0
        # weights vs the rest; tile0's pd half vs pa half).
        tgtT0_sb = persist.tile([P, KC, P], fp8, name="tgtT0_sb", tag="tgtT0_sb")
        tgtTr_sb = persist.tile([P, KC, JS - P], fp8, name="tgtTr_sb", tag="tgtTr_sb")
        negtg_sb = persist.tile([P, JT], f32, name="negtg_sb", tag="negtg_sb")
        inT0pd_sb = persist.tile([P, KC, DW], fp8, name="inT0pd", tag="inT0pd")
        inT0pa_sb = persist.tile([P, KC, CS], fp8, name="inT0pa", tag="inT0pa")
        inT_sb = [None] + [
            persist.tile([P, KC, IBW], fp8, name=f"inT_{ib}", tag=f"inT_{ib}")
            for ib in range(1, IB)
        ]
        acc = [
            persist.tile([P, DW], fp16, name=f"acc{ib}", tag=f"acc{ib}")
            for ib in range(IB)
        ]

        # --- loads: first wave unblocks (jt0, ib0); spread across the three
        # DMA-dispatch queues so everything lands within ~6us ---
        # scalar's HWDGE queue finishes boot ~1us before sync's; put the
        # critical first-wave there: jt0 weights + tile0's DVE half + inT1
        nc.scalar.dma_start(out=tgtT0_sb[:, 0, :], in_=tgtT_d[0:P, 0:P])
        nc.scalar.dma_start(out=tgtT0_sb[:, 1, :], in_=tgtT_d[P : 2 * P, 0:P])
        nc.scalar.dma_start(out=inT0pd_sb[:, 0, :], in_=inT_d[0:P, CS:IBW])
        nc.scalar.dma_start(out=inT0pd_sb[:, 1, :], in_=inT_d[P : 2 * P, CS:IBW])
        nc.sync.dma_start(out=inT0pa_sb[:, 0, :], in_=inT_d[0:P, 0:CS])
        nc.sync.dma_start(out=inT0pa_sb[:, 1, :], in_=inT_d[P : 2 * P, 0:CS])
        nc.sync.dma_start(out=negtg_sb[:], in_=negtg_d[:, :])
        nc.scalar.dma_start(out=inT_sb[1][:, 0, :], in_=inT_d[0:P, IBW : 2 * IBW])
        nc.scalar.dma_start(out=inT_sb[1][:, 1, :], in_=inT_d[P : 2 * P, IBW : 2 * IBW])
        nc.sync.dma_start(out=inT_sb[2][:, 0, :], in_=inT_d[0:P, 2 * IBW : 3 * IBW])
        nc.sync.dma_start(out=inT_sb[2][:, 1, :], in_=inT_d[P : 2 * P, 2 * IBW : 3 * IBW])
        # keep gpsimd free of DMAs entirely: its SWDGE quiesce DRAIN
        # (~3.7us) sits in the NEFF epilogue
        nc.scalar.dma_start(out=inT_sb[3][:, 0, :], in_=inT_d[0:P, 3 * IBW : 4 * IBW])
        nc.scalar.dma_start(out=inT_sb[3][:, 1, :], in_=inT_d[P : 2 * P, 3 * IBW : 4 * IBW])
        nc.sync.dma_start(out=tgtTr_sb[:, 0, :], in_=tgtT_d[0:P, P:JS])
        nc.sync.dma_start(out=tgtTr_sb[:, 1, :], in_=tgtT_d[P : 2 * P, P:JS])

        # init accumulators to fp16 lowest on the (otherwise idle) Pool
        # engine so every tile takes the fused STT path on DVE -- no
        # separate init drains, no startup serialization.  After the gpsimd
        # load dispatches so they don't delay inT3.
        for ib in range(IB):
            nc.gpsimd.memset(acc[ib][:], -65504.0)

        for jt in range(JT):
            for ib in range(IB):
                t = jt * IB + ib
                pa = psum_a.tile([P, CS], f32)
                pd = psum_d.tile([P, DW], f32)
                if jt == 0:
                    wts = tgtT0_sb[:, :, :]
                else:
                    wts = tgtTr_sb[:, :, (jt - 1) * P : jt * P]
                if ib == 0:
                    rhs_pd = [inT0pd_sb[:, :, h * NT : (h + 1) * NT] for h in range(DW // NT)]
                    rhs_pa = [inT0pa_sb[:, :, h * NT : (h + 1) * NT] for h in range(CS // NT)]
                else:
                    rhs_pd = [
                        inT_sb[ib][:, :, CS + h * NT : CS + (h + 1) * NT]
                        for h in range(DW // NT)
                    ]
                    rhs_pa = [
                        inT_sb[ib][:, :, h * NT : (h + 1) * NT] for h in range(CS // NT)
                    ]
                for h in range(DW // NT):
                    nc.tensor.matmul(
                        pd[:, h * NT : (h + 1) * NT],
                        wts,
                        rhs_pd[h],
                        start=True,
                        stop=True,
                        perf_mode=mybir.MatmulPerfMode.DoubleRow,
                    )
                for h in range(CS // NT):
                    nc.tensor.matmul(
                        pa[:, h * NT : (h + 1) * NT],
                        wts,
                        rhs_pa[h],
                        start=True,
                        stop=True,
                        perf_mode=mybir.MatmulPerfMode.DoubleRow,
                    )
                bias_col = negtg_sb[:, jt : jt + 1]
                # ACT: drain cols [0:CS) to fp16 staging, ship raw to HBM
                m_t = m_pool.tile([P, CS], fp16)
                nc.scalar.activation(
                    out=m_t[:],
                    in_=pa[:],
                    func=mybir.ActivationFunctionType.Identity,
                    bias=bias_col,
                    scale=1.0,
                )
                nc.sync.dma_start(out=ship_d[:, t * CS : (t + 1) * CS], in_=m_t[:])
                # DVE: drain cols [CS:IBW) fused bias+max into acc
                nc.vector.scalar_tensor_tensor(
                    out=acc[ib][:],
                    in0=pd[:],
                    scalar=bias_col,
                    in1=acc[ib][:],
                    op0=mybir.AluOpType.add,
                    op1=mybir.AluOpType.max,
                )
                if jt == JT - 1:
                    nc.sync.dma_start(
                        out=out_d[:, ib * DW : (ib + 1) * DW], in_=acc[ib][:]
                    )

    nc.compile()
    return nc


def _get_nc():
    if "nc" not in _CACHE:
        _CACHE["nc"] = _build_nc()
    return _CACHE["nc"]


LAST_RESULTS = None  # BassKernelResults of the most recent run (for test harness)


def _install_ntff_hook_shim():
    """The image's antenv lacks axon_hooks; register an equivalent module so
    run_bass_kernel_spmd(trace=True) can capture NTFF profiles via the axon
    ctypes path.  Harmless when tracing is off."""
    import types

    try:
        import antenv.axon_hooks  # noqa: F401

        return
    except ImportError:
        pass
    hook = None
    try:
        from trn_agent_boot.trn_boot import _ntff_profile_via_ctypes

        hook = _ntff_profile_via_ctypes("/opt/axon/libaxon_pjrt.so")
    except Exception:
        pass
    try:
        import antenv

        mod = types.ModuleType("antenv.axon_hooks")
        mod.get_axon_ntff_profile_hook = lambda: hook
        mod.set_axon_ntff_profile_hook = lambda h: None
        sys.modules["antenv.axon_hooks"] = mod
        antenv.axon_hooks = mod
    except Exception:
        pass


def kernel(input, target):
    global LAST_RESULTS
    from concourse.bass_utils import run_bass_kernel_spmd

    _install_ntff_hook_shim()

    nc = _get_nc()

    inp = np.asarray(input, dtype=np.float32)
    tgt = np.asarray(target, dtype=np.float32)
    assert inp.shape == (B, D) and tgt.shape == (B, D)

    tgtT_full = np.ascontiguousarray(tgt.T).astype(ml_dtypes.float8_e4m3)  # [D, B]
    inT_np = np.ascontiguousarray((2.0 * inp).T).astype(ml_dtypes.float8_e4m3)  # [D, B]
    tgsq = np.sum(tgt.astype(np.float64) ** 2, axis=1)
    C = float(tgsq.mean())
    tgsqc = -(tgsq - C).astype(np.float32)  # negated, centered

    in_maps = [
        {
            "tgtT": np.ascontiguousarray(tgtT_full[:, c * JS : (c + 1) * JS]),
            "inT": inT_np,
            "negtg": np.ascontiguousarray(
                tgsqc[c * JS : (c + 1) * JS].reshape(JT, P).T
            ),
        }
        for c in range(NCORES)
    ]

    trace = bool(int(os.environ.get("KERNEL_TRACE", "0")))
    res = run_bass_kernel_spmd(nc, in_maps, core_ids=list(range(NCORES)), trace=trace)
    LAST_RESULTS = res

    # Reassemble per-core partials [128, B]: acc covers the DVE column ranges
    # (i in [ib*IBW+CS, (ib+1)*IBW)), shipped tiles cover the ACT ranges.
    rowmax_all = np.full((P, B), -np.inf, dtype=np.float32)
    for c in range(NCORES):
        r = res.results[c]
        accp = r["rowmax"].astype(np.float32)  # [P, IB*DW]
        ship = r["ship"].astype(np.float32)  # [P, JT*IB*CS]
        for ib in range(IB):
            lo = ib * IBW + CS
            np.maximum(
                rowmax_all[:, lo : lo + DW],
                accp[:, ib * DW : (ib + 1) * DW],
                out=rowmax_all[:, lo : lo + DW],
            )
            for jt in range(JT):
                t = jt * IB + ib
                lo2 = ib * IBW
                np.maximum(
                    rowmax_all[:, lo2 : lo2 + CS],
                    ship[:, t * CS : (t + 1) * CS],
                    out=rowmax_all[:, lo2 : lo2 + CS],
                )
    rowmin = -rowmax_all.max(axis=0)  # [B]
    in_sq = np.sum(inp.astype(np.float64) ** 2, axis=1)
    loss = np.mean((in_sq + C + rowmin.astype(np.float64)) / float(D))
    return np.asarray(loss, dtype=np.float32)


# revision 57
# speedup vs baseline: 1.2187x; 1.2187x over previous
"""Trainium2 Bass kernel for BatchMatchedMSELoss.

loss = mean_i min_j mean_d (input[i,d] - target[j,d])^2

Decomposition:
  mse[i,j]  = (||x_i||^2 + ||t_j||^2 - 2<x_i, t_j>) / D
  min_j mse = (||x_i||^2 + min_j(||t_j||^2 - 2<x_i,t_j>)) / D

Sharding: each core owns a 1024-row slice of TARGET (j) and sees ALL input
rows i; j lives on PSUM partitions:
  psum[jp, i] = 2<x_i, t_j>      (fp8 e4m3 DoubleRow matmul, K=256, f32 accum)

The kernel is PSUM-drain-bound: only DVE and ACT can read PSUM (~1.04-1.24
ns/col/op measured).  Each [128, 2048] tile of the cross matrix is drained
CONCURRENTLY by both engines on disjoint 2-bank psum halves allocated from
SEPARATE tile pools (distinct tile objects keep the Tile framework from
serializing the two consumers):
  ACT   cols [0:1024):    activation identity+bias -> fp16 staging, shipped
                          raw to HBM; the host max-folds shipped tiles.
  DVE   cols [1024:2048): fused scalar_tensor_tensor (bias+max into acc).
The DVE half is produced first (its stream is the long pole).  Both engines
run back-to-back at ~1.0-1.2us per tile; steady state ~= 32 tiles x ~1.2us.
Host combines acc partials and shipped tiles, maxes over (core, partition),
adds ||x_i||^2 + C, /D, and means.

Measured on trn2 (8 cores, axon): HW exec ~62-64us (baseline 91us), loss
relative error vs the f32 reference ~1.2e-4.
"""

import os
import sys

sys.path.insert(0, "/opt/trn_rl_repo")

import numpy as np
import ml_dtypes

B = 8192
D = 256
NCORES = 8
JS = B // NCORES  # 1024 target rows (j) per core
P = 128
KC = D // P  # 2 contraction chunks
JT = JS // P  # 8 j-tiles per core
NT = 512  # PSUM bank width in f32
IBW = 2048  # i-block width (one PSUM tile = 4 banks, 2 bufs)
IB = B // IBW  # 4 i-blocks
HS = IBW // NT  # 4 matmul subtiles per (j-tile, i-block)
CS = 1024  # ACT-drained column split: bank-aligned 2+2 banks per engine
DW = IBW - CS  # DVE-drained width (1024)

_CACHE = {}


def _build_nc():
    from contextlib import ExitStack

    import concourse.bacc as bacc
    import concourse.tile as tile
    import concourse.mybir as mybir

    fp16 = mybir.dt.float16
    f32 = mybir.dt.float32

    nc = bacc.Bacc("TRN2", target_bir_lowering=False, debug=False)

    fp8 = mybir.dt.float8e4

    # tgtT/inT are host-pre-arranged partition-major [P, KC, cols] so each
    # SBUF tile loads with ONE 3D DMA (halves the dispatch-chain latency)
    tgtT_d = nc.dram_tensor("tgtT", [P, KC, JS], fp8, kind="ExternalInput").ap()
    inT_d = nc.dram_tensor("inT", [P, KC, B], fp8, kind="ExternalInput").ap()
    # negtg[p, jt] = -(||t_j||^2 - C) for local j = jt*128 + p
    negtg_d = nc.dram_tensor("negtg", [P, JT], f32, kind="ExternalInput").ap()
    # acc partials over the DVE column ranges; host maxes over (core, p)
    out_d = nc.dram_tensor("rowmax", [P, IB * DW], fp16, kind="ExternalOutput").ap()
    # raw biased fp16 tiles (ACT ranges), host-folded: JT*IB slots of [P, CS]
    ship_d = nc.dram_tensor("ship", [P, JT * IB * CS], fp16, kind="ExternalOutput").ap()

    with tile.TileContext(nc) as tc, ExitStack() as ctx:
        persist = ctx.enter_context(tc.tile_pool(name="persist", bufs=1))
        # separate pools for the ACT-drained and DVE-drained halves of each
        # tile: distinct tile objects prevent the Tile framework from
        # serializing the two consumers of one psum tile
        psum_a = ctx.enter_context(tc.tile_pool(name="psum_a", bufs=2, space="PSUM"))
        psum_d = ctx.enter_context(tc.tile_pool(name="psum_d", bufs=2, space="PSUM"))
        m_pool = ctx.enter_context(tc.tile_pool(name="m", bufs=8))

        # --- persistent SBUF buffers ---
        # NOTE: Tile's DMA-write tracking is per-tile-object, not per-range:
        # a reader waits for ALL loads into the tile it touches.  Regions
        # consumed at different times therefore get separate tiles (jt0
        # weights vs the rest; tile0's pd half vs pa half).
        tgtT0_sb = persist.tile([P, KC, P], fp8, name="tgtT0_sb", tag="tgtT0_sb")
        tgtTr_sb = persist.tile([P, KC, JS - P], fp8, name="tgtTr_sb", tag="tgtTr_sb")
        negtg_sb = persist.tile([P, JT], f32, name="negtg_sb", tag="negtg_sb")
        inT0pd_sb = persist.tile([P, KC, DW], fp8, name="inT0pd", tag="inT0pd")
        inT0pa_sb = persist.tile([P, KC, CS], fp8, name="inT0pa", tag="inT0pa")
        inT_sb = [None] + [
            persist.tile([P, KC, IBW], fp8, name=f"inT_{ib}", tag=f"inT_{ib}")
            for ib in range(1, IB)
        ]
        acc = [
            persist.tile([P, DW], fp16, name=f"acc{ib}", tag=f"acc{ib}")
            for ib in range(IB)
        ]

        # --- loads: first wave unblocks (jt0, ib0); spread across the three
        # DMA-dispatch queues so everything lands within ~6us ---
        # scalar's HWDGE queue finishes boot ~1us before sync's; put the
        # critical first-wave there: jt0 weights + tile0's DVE half + inT1
        nc.scalar.dma_start(out=tgtT0_sb[:], in_=tgtT_d[:, :, 0:P])
        nc.scalar.dma_start(out=inT0pd_sb[:], in_=inT_d[:, :, CS:IBW])
        nc.sync.dma_start(out=inT0pa_sb[:], in_=inT_d[:, :, 0:CS])
        nc.sync.dma_start(out=negtg_sb[:], in_=negtg_d[:, :])
        nc.scalar.dma_start(out=inT_sb[1][:], in_=inT_d[:, :, IBW : 2 * IBW])
        nc.sync.dma_start(out=inT_sb[2][:], in_=inT_d[:, :, 2 * IBW : 3 * IBW])
        # keep gpsimd free of DMAs entirely: its SWDGE quiesce DRAIN
        # (~3.7us) sits in the NEFF epilogue
        nc.scalar.dma_start(out=inT_sb[3][:], in_=inT_d[:, :, 3 * IBW : 4 * IBW])
        nc.sync.dma_start(out=tgtTr_sb[:], in_=tgtT_d[:, :, P:JS])

        # init accumulators to fp16 lowest on the (otherwise idle) Pool
        # engine so every tile takes the fused STT path on DVE -- no
        # separate init drains, no startup serialization.  After the gpsimd
        # load dispatches so they don't delay inT3.
        for ib in range(IB):
            nc.gpsimd.memset(acc[ib][:], -65504.0)

        for jt in range(JT):
            for ib in range(IB):
                t = jt * IB + ib
                pa = psum_a.tile([P, CS], f32)
                pd = psum_d.tile([P, DW], f32)
                if jt == 0:
                    wts = tgtT0_sb[:, :, :]
                else:
                    wts = tgtTr_sb[:, :, (jt - 1) * P : jt * P]
                if ib == 0:
                    rhs_pd = [inT0pd_sb[:, :, h * NT : (h + 1) * NT] for h in range(DW // NT)]
                    rhs_pa = [inT0pa_sb[:, :, h * NT : (h + 1) * NT] for h in range(CS // NT)]
                else:
                    rhs_pd = [
                        inT_sb[ib][:, :, CS + h * NT : CS + (h + 1) * NT]
                        for h in range(DW // NT)
                    ]
                    rhs_pa = [
                        inT_sb[ib][:, :, h * NT : (h + 1) * NT] for h in range(CS // NT)
                    ]
                for h in range(DW // NT):
                    nc.tensor.matmul(
                        pd[:, h * NT : (h + 1) * NT],
                        wts,
                        rhs_pd[h],
                        start=True,
                        stop=True,
                        perf_mode=mybir.MatmulPerfMode.DoubleRow,
                    )
                for h in range(CS // NT):
                    nc.tensor.matmul(
                        pa[:, h * NT : (h + 1) * NT],
                        wts,
                        rhs_pa[h],
                        start=True,
                        stop=True,
                        perf_mode=mybir.MatmulPerfMode.DoubleRow,
                    )
                bias_col = negtg_sb[:, jt : jt + 1]
                # ACT: drain cols [0:CS) to fp16 staging, ship raw to HBM
                m_t = m_pool.tile([P, CS], fp16)
                nc.scalar.activation(
                    out=m_t[:],
                    in_=pa[:],
                    func=mybir.ActivationFunctionType.Identity,
                    bias=bias_col,
                    scale=1.0,
                )
                nc.sync.dma_start(out=ship_d[:, t * CS : (t + 1) * CS], in_=m_t[:])
                # DVE: drain cols [CS:IBW) fused bias+max into acc
                nc.vector.scalar_tensor_tensor(
                    out=acc[ib][:],
                    in0=pd[:],
                    scalar=bias_col,
                    in1=acc[ib][:],
                    op0=mybir.AluOpType.add,
                    op1=mybir.AluOpType.max,
                )
                if jt == JT - 1:
                    nc.sync.dma_start(
                        out=out_d[:, ib * DW : (ib + 1) * DW], in_=acc[ib][:]
                    )

    nc.compile()
    return nc


def _get_nc():
    if "nc" not in _CACHE:
        _CACHE["nc"] = _build_nc()
    return _CACHE["nc"]


LAST_RESULTS = None  # BassKernelResults of the most recent run (for test harness)


def _install_ntff_hook_shim():
    """The image's antenv lacks axon_hooks; register an equivalent module so
    run_bass_kernel_spmd(trace=True) can capture NTFF profiles via the axon
    ctypes path.  Harmless when tracing is off."""
    import types

    try:
        import antenv.axon_hooks  # noqa: F401

        return
    except ImportError:
        pass
    hook = None
    try:
        from trn_agent_boot.trn_boot import _ntff_profile_via_ctypes

        hook = _ntff_profile_via_ctypes("/opt/axon/libaxon_pjrt.so")
    except Exception:
        pass
    try:
        import antenv

        mod = types.ModuleType("antenv.axon_hooks")
        mod.get_axon_ntff_profile_hook = lambda: hook
        mod.set_axon_ntff_profile_hook = lambda h: None
        sys.modules["antenv.axon_hooks"] = mod
        antenv.axon_hooks = mod
    except Exception:
        pass


def kernel(input, target):
    global LAST_RESULTS
    from concourse.bass_utils import run_bass_kernel_spmd

    _install_ntff_hook_shim()

    nc = _get_nc()

    inp = np.asarray(input, dtype=np.float32)
    tgt = np.asarray(target, dtype=np.float32)
    assert inp.shape == (B, D) and tgt.shape == (B, D)

    tgtT_full = np.ascontiguousarray(tgt.T).astype(ml_dtypes.float8_e4m3)  # [D, B]
    inT_np = np.ascontiguousarray((2.0 * inp).T).astype(ml_dtypes.float8_e4m3)  # [D, B]
    tgsq = np.sum(tgt.astype(np.float64) ** 2, axis=1)
    C = float(tgsq.mean())
    tgsqc = -(tgsq - C).astype(np.float32)  # negated, centered

    def pmaj(a):  # [D, cols] -> [P, KC, cols] partition-major
        return np.ascontiguousarray(a.reshape(KC, P, a.shape[1]).swapaxes(0, 1))

    inT_pm = pmaj(inT_np)
    in_maps = [
        {
            "tgtT": pmaj(tgtT_full[:, c * JS : (c + 1) * JS]),
            "inT": inT_pm,
            "negtg": np.ascontiguousarray(
                tgsqc[c * JS : (c + 1) * JS].reshape(JT, P).T
            ),
        }
        for c in range(NCORES)
    ]

    trace = bool(int(os.environ.get("KERNEL_TRACE", "0")))
    res = run_bass_kernel_spmd(nc, in_maps, core_ids=list(range(NCORES)), trace=trace)
    LAST_RESULTS = res

    # Reassemble per-core partials [128, B]: acc covers the DVE column ranges
    # (i in [ib*IBW+CS, (ib+1)*IBW)), shipped tiles cover the ACT ranges.
    rowmax_all = np.full((P, B), -np.inf, dtype=np.float32)
    for c in range(NCORES):
        r = res.results[c]
        accp = r["rowmax"].astype(np.float32)  # [P, IB*DW]
        ship = r["ship"].astype(np.float32)  # [P, JT*IB*CS]
        for ib in range(IB):
            lo = ib * IBW + CS
            np.maximum(
                rowmax_all[:, lo : lo + DW],
                accp[:, ib * DW : (ib + 1) * DW],
                out=rowmax_all[:, lo : lo + DW],
            )
            for jt in range(JT):
                t = jt * IB + ib
                lo2 = ib * IBW
                np.maximum(
                    rowmax_all[:, lo2 : lo2 + CS],
                    ship[:, t * CS : (t + 1) * CS],
                    out=rowmax_all[:, lo2 : lo2 + CS],
                )
    rowmin = -rowmax_all.max(axis=0)  # [B]
    in_sq = np.sum(inp.astype(np.float64) ** 2, axis=1)
    loss = np.mean((in_sq + C + rowmin.astype(np.float64)) / float(D))
    return np.asarray(loss, dtype=np.float32)
